# revision 1
# baseline (speedup 1.0000x reference)
"""Trainium2 Bass kernel for the vq_codebook problem.

Computes, per batch b (B=32, d=512, n=4096, r=64, T=10, 3 steps):
    D = normalize(D_init, dim=d)
    repeat 3x: Dn = normalize(D); cos = Dn^T @ normalize(X, dim=d);
               C = softmax(cos / T, over r); D = X @ C^T   (normalize-invariant
               scale factors like the per-codeword count division cancel)
    Xbar = normalize(D) @ C of the last step.

Sharding: pure batch parallelism, 4 batches per NeuronCore across 8 cores.

Layout strategy per batch:
  - X loaded natural [d, n]; PE-transposed once to XT [n, d] for the
    n-contraction (XCt); cast to bf16 for the d-contraction (cos).
  - All softmax work happens in the transposed [n-on-partitions, r-free]
    layout where the 1/||x_n|| logit scale and the softmax denominator are
    per-partition/free-dim ops.
  - Matmuls run in bf16 with fp32 PSUM accumulation; cos and XCt use
    tile_position col-tiling so two 64-wide matmuls share the PE array.
    Measured rel err vs the fp32 reference: ~3e-3.
"""

import numpy as np

import concourse.bacc as bacc
import concourse.bass as bass
import concourse.mybir as mybir
import concourse.tile as tile
from concourse.bass_utils import run_bass_kernel_spmd

F32 = mybir.dt.float32
F32R = mybir.dt.float32r
BF16 = mybir.dt.bfloat16
AF = mybir.ActivationFunctionType
OP = mybir.AluOpType

N_CORES = 8
B_FULL, D, N, R = 32, 512, 4096, 64
B_LOC = B_FULL // N_CORES          # 4 batches per core
KT = D // 128                      # 4 d-tiles
NC128 = N // 128                   # 32 n-chunks of 128
NB512 = N // 512                   # 8 n-blocks of 512
NG = NC128 // 8                    # 4 groups of 8 chunks (512 n each)
T_INV = 0.1                        # 1 / temperature
STEPS = 3
EPS2 = 1e-12                       # eps^2 for the norm clamp


def _bcast(ap_2d, free_rep):
    """View a [P, m] AP as [P, m, free_rep] with stride-0 inner dim."""
    return bass.AP(
        tensor=ap_2d.tensor,
        offset=ap_2d.offset,
        ap=[ap_2d.ap[0], list(ap_2d.ap[1]), [0, free_rep]],
    )


def _rsqrt_clamped(nc, pool, src_ap, p, name, eps_t):
    """exp(-0.5 * ln(src + EPS2)) as an [p, m] tile; src_ap is [p, m].

    The additive EPS2 inside the Ln replaces max(src, EPS2): identical for
    src >> EPS2 (always, here) and still a safe floor at src ~ 0, while
    saving a DVE hop on the serial normalize chain."""
    m = src_ap.shape[1]
    ln = pool.tile([p, m], F32, tag=f"{name}_ln")
    nc.scalar.activation(out=ln, in_=src_ap, func=AF.Ln, scale=1.0,
                         bias=eps_t[:p, 0:1])
    rs = pool.tile([p, m], F32, tag=f"{name}_rs")
    nc.scalar.activation(out=rs, in_=ln, func=AF.Exp, scale=-0.5, bias=0.0)
    return rs


def _force_single_act_set():
    """All ACT functions we use (Exp, Ln, Square, Copy) live in the
    natural_log_exp_and_others set.  The table-load pass first-matches each
    function against the set list, which alternates loads between two sets
    (~1.3 us each).  Empty out every other set (ids keep their positions) so
    everything resolves to the one set and a single load suffices."""
    import concourse.hw_specs as hw_specs

    orig = hw_specs.get_activation_tables
    target = "natural_log_exp_and_others"

    def patched(arch):
        t = dict(orig(arch))
        need = {AF.Exp, AF.Ln, AF.Square, AF.Copy}
        if target in t and need <= set(t[target]):
            t = {k: (v if k == target else set()) for k, v in t.items()}
        return t

    bacc.get_activation_tables = patched


def build_program():
    _force_single_act_set()
    nc = bacc.Bacc()
    x_ext = nc.declare_dram_parameter("X", [B_LOC, D, N], F32, isOutput=False)
    d_ext = nc.declare_dram_parameter("Dinit", [B_LOC, D, R], F32, isOutput=False)
    id_ext = nc.declare_dram_parameter("ident", [128, 128], F32, isOutput=False)
    y_ext = nc.declare_dram_parameter("Y", [B_LOC, D, N], F32, isOutput=True)

    with tile.TileContext(nc) as tc:
        import contextlib

        with contextlib.ExitStack() as ctx:
            singles = ctx.enter_context(tc.tile_pool(name="singles", bufs=1))
            xpool = ctx.enter_context(tc.tile_pool(name="xpool", bufs=1))
            xnat = ctx.enter_context(tc.tile_pool(name="xnat", bufs=8))
            work = ctx.enter_context(tc.tile_pool(name="work", bufs=2))
            work3 = ctx.enter_context(tc.tile_pool(name="work3", bufs=8))
            dpool = ctx.enter_context(tc.tile_pool(name="dpool", bufs=2))
            ps_big = ctx.enter_context(tc.tile_pool(name="ps_big", bufs=3, space="PSUM"))
            ps_cos = ctx.enter_context(tc.tile_pool(name="ps_cos", bufs=2, space="PSUM"))
            ps_ct = ctx.enter_context(tc.tile_pool(name="ps_ct", bufs=1, space="PSUM"))
            ps_acc = ctx.enter_context(tc.tile_pool(name="ps_acc", bufs=2, space="PSUM"))

            # identities in the three matmul dtypes
            id_f = singles.tile([128, 128], F32)
            nc.sync.dma_start(out=id_f, in_=id_ext[:])
            id_b = singles.tile([128, 128], BF16)
            nc.vector.tensor_copy(out=id_b, in_=id_f)
            eps_t = singles.tile([128, 1], F32)
            nc.vector.memset(eps_t, EPS2)

            for b in range(B_LOC):
                # ---------------- setup: load X, transpose, cast, norms ------
                xbf = [xpool.tile([128, N], BF16, tag=f"xbf{k}", name=f"xbf{k}") for k in range(KT)]
                xt = [xpool.tile([128, D], BF16, tag=f"xt{c}", name=f"xt{c}", bufs=2) for c in range(NC128)]
                ssq = xpool.tile([128, NC128], F32, tag="ssq")

                for h in range(4):  # quarters of n
                    xn_h = []
                    for k in range(KT):
                        t = xnat.tile([128, N // 4], F32, tag="xnat")
                        nc.sync.dma_start(
                            out=t,
                            in_=x_ext[b, k * 128:(k + 1) * 128,
                                      h * (N // 4):(h + 1) * (N // 4)],
                        )
                        xn_h.append(t)
                        nc.vector.tensor_copy(
                            out=xbf[k][:, h * (N // 4):(h + 1) * (N // 4)], in_=t
                        )
                    for ci in range(NC128 // 4):
                        c = h * (NC128 // 4) + ci
                        pt = ps_big.tile([128, D], F32, tag="pbig")
                        for k in range(KT):
                            nc.tensor.transpose(
                                pt[:, k * 128:(k + 1) * 128],
                                xn_h[k][:, ci * 128:(ci + 1) * 128],
                                id_f,
                            )
                        nc.vector.tensor_copy(out=xt[c], in_=pt)
                        sq = ps_ct.tile([128, D], F32, tag="pct")
                        nc.scalar.activation(
                            out=sq, in_=pt, func=AF.Square, scale=1.0, bias=0.0,
                            accum_out=ssq[:, c:c + 1],
                        )
                # scl[p, c] = 1 / max(||x_n||, eps), n = c*128 + p
                ln_x = work.tile([128, NC128], F32, tag="sclw_ln")
                nc.scalar.activation(out=ln_x, in_=ssq[:, :], func=AF.Ln,
                                     scale=1.0, bias=eps_t[:, 0:1])
                scl = xpool.tile([128, NC128], F32, tag="scl")
                nc.scalar.activation(out=scl, in_=ln_x, func=AF.Exp,
                                     scale=-0.5, bias=0.0)

                # D_init^T: load natural, transpose to DT [64, 512]
                dt_cur = dpool.tile([64, D], F32, tag="dt")
                pdn = ps_cos.tile([64, 512], F32, tag="pcos")
                for k in range(KT):
                    dn_nat = work.tile([128, R], F32, tag="dload")
                    nc.sync.dma_start(
                        out=dn_nat, in_=d_ext[b, k * 128:(k + 1) * 128, :]
                    )
                    nc.tensor.transpose(
                        pdn[:, k * 128:(k + 1) * 128], dn_nat, id_f
                    )
                nc.scalar.copy(out=dt_cur, in_=pdn)

                # ---------------- 3 VQ steps --------------------------------
                for s in range(STEPS):
                    last = s == STEPS - 1
                    # normalize D columns (rows of DT) -> DnT, transpose -> Dn (bf16)
                    dscr = ps_cos.tile([64, D], F32, tag="pcos")
                    ssqd = work.tile([64, 1], F32, tag="ssqd")
                    nc.vector.scalar_tensor_tensor(
                        out=dscr, in0=dt_cur, scalar=1.0, in1=dt_cur,
                        op0=OP.mult, op1=OP.mult, accum_out=ssqd,
                    )
                    rnd = _rsqrt_clamped(nc, work, ssqd[:, :], 64, "rnd", eps_t)
                    dnt = work.tile([64, D], F32, tag="dnt")
                    nc.vector.tensor_scalar_mul(out=dnt, in0=dt_cur, scalar1=rnd)
                    dn_bf = work.tile([128, KT, R], BF16, tag="dnbf")
                    pdn2 = ps_big.tile([128, KT * R], F32, tag="pbig")
                    for k in range(KT):
                        nc.tensor.transpose(
                            pdn2[:, k * R:(k + 1) * R],
                            dnt[:, k * 128:(k + 1) * 128], id_f[0:64, 0:64],
                        )
                    nc.scalar.copy(out=dn_bf, in_=pdn2.rearrange("p (k r) -> p k r", k=KT))

                    # cos blocks (col-tiled pairs), packed E-transposes,
                    # softmax, CT.  Pair g covers n-blocks 2g (top half of
                    # the psum tile) and 2g+1 (bottom half).
                    ct_g = []
                    for g in range(NG):
                        pct = ps_ct.tile([128, 4, 128], BF16, tag="pct")
                        pc2 = ps_cos.tile([128, 512], F32, tag="pcos")
                        j0, j1 = 2 * g, 2 * g + 1
                        for k in range(KT):
                            nc.tensor.matmul(
                                pc2[0:64, :], dn_bf[:, k, :],
                                xbf[k][:, j0 * 512:(j0 + 1) * 512],
                                start=(k == 0), stop=(k == KT - 1),
                                tile_position=(0, 0),
                            )
                            nc.tensor.matmul(
                                pc2[64:128, :], dn_bf[:, k, :],
                                xbf[k][:, j1 * 512:(j1 + 1) * 512],
                                start=(k == 0), stop=(k == KT - 1),
                                tile_position=(0, 64), skip_group_check=True,
                            )
                        cos_sb = work3.tile([128, 512], BF16, tag="cossb")
                        nc.scalar.copy(out=cos_sb, in_=pc2)
                        # One full 128x128 transpose flips a [2*r, n128]
                        # block: out columns 0:64 = cosT of block j0,
                        # 64:128 = cosT of block j1 (both at this n-chunk).
                        for ci in range(4):
                            nc.tensor.transpose(
                                pct[:, ci, :],
                                cos_sb[:, ci * 128:(ci + 1) * 128],
                                id_b,
                            )
                        # scale order along the packed axis: (ci, half) ->
                        # chunk (2g+half)*4+ci = scl column 8g + 4*half + ci
                        scl_s = scl[:, 8 * g:8 * (g + 1)]
                        scl_v = bass.AP(
                            tensor=scl_s.tensor, offset=scl_s.offset,
                            ap=[list(scl_s.ap[0]), [1, 4], [4, 2], [0, R]],
                        )
                        pct_v = pct.rearrange("p c (h r) -> p c h r", h=2)
                        logits = work3.tile([128, 4, 2, R], BF16, tag="logits")
                        nc.vector.tensor_tensor(
                            out=logits, in0=pct_v, in1=scl_v, op=OP.mult,
                        )
                        et = work3.tile([128, 4, 2, R], BF16, tag="et")
                        nc.scalar.activation(
                            out=et, in_=logits, func=AF.Exp, scale=T_INV, bias=0.0
                        )
                        s_sum = work3.tile([128, 4, 2], F32, tag="ssum")
                        nc.vector.tensor_reduce(
                            out=s_sum, in_=et, axis=mybir.AxisListType.X, op=OP.add
                        )
                        rs_sum = work3.tile([128, 4, 2], F32, tag="rssum")
                        nc.vector.reciprocal(out=rs_sum, in_=s_sum)
                        rs_b = bass.AP(
                            tensor=rs_sum.tensor, offset=rs_sum.offset,
                            ap=[list(rs_sum.ap[0]), [2, 4], [1, 2], [0, R]],
                        )
                        ct = work.tile([128, 4, 2, R], BF16, tag="ct", bufs=4, name=f"ct{g}")
                        nc.vector.tensor_tensor(
                            out=ct, in0=et, in1=rs_b, op=OP.mult
                        )
                        ct_g.append(ct)

                    # XCt^T [r=64, d=512]: bf16 col-tiled pairs — even chunks
                    # accumulate into partitions 0-63, odd into 64-127,
                    # halves summed after.  ct chunk for global chunk c is
                    # ct_g[c//8][:, c%4, (c%8)//4, :].
                    def ct_chunk(c):
                        return ct_g[c // 8][:, c % 4, (c % 8) // 4, :]

                    pacc = ps_acc.tile([128, D], F32, tag="pacc")
                    for cp in range(NC128 // 2):
                        ca, cb = 2 * cp, 2 * cp + 1
                        nc.tensor.matmul(
                            pacc[0:64, :], ct_chunk(ca), xt[ca],
                            start=(cp == 0), stop=(cp == NC128 // 2 - 1),
                            tile_position=(0, 0),
                        )
                        nc.tensor.matmul(
                            pacc[64:128, :], ct_chunk(cb), xt[cb],
                            start=(cp == 0), stop=(cp == NC128 // 2 - 1),
                            tile_position=(0, 64), skip_group_check=True,
                        )
                    xct_half = work.tile([64, D], F32, tag="xcthalf")
                    nc.scalar.copy(out=xct_half, in_=pacc[0:64, :])

                    if not last:
                        dt_cur = dpool.tile([64, D], F32, tag="dt")
                        nc.vector.tensor_tensor(
                            out=dt_cur, in0=xct_half, in1=pacc[64:128, :],
                            op=OP.add,
                        )
                    else:
                        # Dnew^T normalized, in bf16 for the Xbar matmul
                        dnew_f = work.tile([64, D], F32, tag="dnewf")
                        nc.vector.tensor_tensor(
                            out=dnew_f, in0=xct_half, in1=pacc[64:128, :],
                            op=OP.add,
                        )
                        fscr = ps_cos.tile([64, D], F32, tag="pcos")
                        ssqf = work.tile([64, 1], F32, tag="ssqf")
                        nc.vector.scalar_tensor_tensor(
                            out=fscr, in0=dnew_f, scalar=1.0, in1=dnew_f,
                            op0=OP.mult, op1=OP.mult, accum_out=ssqf,
                        )
                        rnf = _rsqrt_clamped(nc, work, ssqf[:, :], 64, "rnf", eps_t)
                        dnew_r = work.tile([64, D], BF16, tag="dnewr")
                        nc.vector.tensor_scalar_mul(
                            out=dnew_r, in0=dnew_f, scalar1=rnf
                        )
                        # C [r=64, n] in bf16 via transposing CT chunks
                        c_r = xpool.tile([64, N], BF16, tag="c_r")
                        for q in range(NB512):
                            pcq = ps_cos.tile([64, 512], BF16, tag="pcos")
                            for ci in range(4):
                                c = q * 4 + ci
                                nc.tensor.transpose(
                                    pcq[:, ci * 128:(ci + 1) * 128],
                                    ct_chunk(c), id_b,
                                )
                            nc.vector.tensor_copy(
                                out=c_r[:, q * 512:(q + 1) * 512], in_=pcq
                            )
                        # Xbar = Dnew @ C
                        for k in range(KT):
                            for j in range(NB512):
                                pxb = ps_big.tile([128, 512], F32, tag="pbig")
                                nc.tensor.matmul(
                                    pxb, dnew_r[:, k * 128:(k + 1) * 128],
                                    c_r[:, j * 512:(j + 1) * 512],
                                    start=True, stop=True,
                                )
                                ot = work3.tile([128, 512], F32, tag="osb")
                                nc.scalar.copy(out=ot, in_=pxb)
                                nc.sync.dma_start(
                                    out=y_ext[b, k * 128:(k + 1) * 128,
                                              j * 512:(j + 1) * 512],
                                    in_=ot,
                                )
    nc.finalize()
    return nc


_NC_CACHE = None
_last_in_maps = None


def kernel(X: np.ndarray, D_init: np.ndarray) -> np.ndarray:
    global _NC_CACHE, _last_in_maps
    X = np.asarray(X, dtype=np.float32)
    D_init = np.asarray(D_init, dtype=np.float32)
    if _NC_CACHE is None:
        _NC_CACHE = build_program()
    nc = _NC_CACHE
    ident = np.eye(128, dtype=np.float32)
    in_maps = [
        {
            "X": np.ascontiguousarray(X[i * B_LOC:(i + 1) * B_LOC]),
            "Dinit": np.ascontiguousarray(D_init[i * B_LOC:(i + 1) * B_LOC]),
            "ident": ident,
        }
        for i in range(N_CORES)
    ]
    _last_in_maps = in_maps
    res = run_bass_kernel_spmd(nc, in_maps, list(range(N_CORES)))
    return np.concatenate([res.results[i]["Y"] for i in range(N_CORES)], axis=0)



# revision 10
# speedup vs baseline: 218.8506x; 218.8506x over previous
"""Trainium2 Bass kernel for the vq_codebook problem.

Computes, per batch b (B=32, d=512, n=4096, r=64, T=10, 3 steps):
    D = normalize(D_init, dim=d)
    repeat 3x: Dn = normalize(D); cos = Dn^T @ normalize(X, dim=d);
               C = softmax(cos / T, over r); D = X @ C^T   (normalize-invariant
               scale factors like the per-codeword count division cancel)
    Xbar = normalize(D) @ C of the last step.

Sharding: pure batch parallelism, 4 batches per NeuronCore across 8 cores.

Layout strategy (per core):
  - Host uploads X twice in the layouts the PE wants: XT = X^T in bf16
    (contraction over n for X@C^T) and X8 = X in fp8-e4m3 packed in
    d-tile pairs (stationary operand of the cos matmul, DoubleRow mode).
    No on-device transposes or casts of X are needed.
  - cos^T is produced directly in the [n-partitions, r-free] layout the
    softmax wants, via fp8 DoubleRow matmuls (X chunk stationary, Dn
    moving) with full 128-partition outputs.
  - X@C^T runs in the natural [d-partitions, r] layout (full-width
    outputs), then a cheap 4x bf16 transpose gives D^T for the
    column-normalize, which is a free-dim reduction there.
  - 1/||x_n|| logit scales come from bf16 X^T via DVE square-accumulate;
    softmax runs batched over all 32 n-chunks at once; the per-codeword
    sum reduction runs on the otherwise-idle GPSIMD engine.
  - Y is produced in bf16 and upcast to f32 on the host after gather.
"""

import contextlib
import math

import numpy as np

import concourse.bacc as bacc
import concourse.bass as bass
import concourse.mybir as mybir
import concourse.tile as tile
from concourse.bass_utils import run_bass_kernel_spmd

F32 = mybir.dt.float32
BF16 = mybir.dt.bfloat16
F8 = mybir.dt.float8e4
AF = mybir.ActivationFunctionType
OP = mybir.AluOpType
DR = mybir.MatmulPerfMode.DoubleRow

N_CORES = 8
B_FULL, D, N, R = 32, 512, 4096, 64
B_LOC = B_FULL // N_CORES          # 4 batches per core
KT = D // 128                      # 4 d-tiles
NC = N // 128                      # 32 n-chunks of 128
T_INV = 0.1                        # 1 / temperature
LN_TINV = math.log(T_INV)
STEPS = 3
EPS2 = 1e-12                       # eps^2 for the norm clamp


def _bcast_mid(ap_2d, g0, g1, rep):
    """View columns [g0:g1) of a [128, m] AP as [128, g1-g0, rep] with a
    stride-0 innermost dim."""
    col_stride = ap_2d.ap[-1][0]
    return bass.AP(
        tensor=ap_2d.tensor,
        offset=ap_2d.offset + g0 * col_stride,
        ap=[list(ap_2d.ap[0]), [col_stride, g1 - g0], [0, rep]],
    )


def _rsqrt_clamped(nc, pool, src_ap, p, name, eps_t, extra_bias=0.0):
    """exp(-0.5 * ln(src + EPS2) + extra_bias) as a [p, 1] tile.

    The additive EPS2 inside the Ln replaces max(src, EPS2): identical for
    src >> EPS2 (always, here) and still a safe floor at src ~ 0."""
    m = src_ap.shape[1]
    ln = pool.tile([p, m], F32, tag=f"{name}_ln")
    nc.scalar.activation(out=ln, in_=src_ap, func=AF.Ln, scale=1.0,
                         bias=eps_t[:p, 0:1])
    rs = pool.tile([p, m], F32, tag=f"{name}_rs")
    nc.scalar.activation(out=rs, in_=ln, func=AF.Exp, scale=-0.5,
                         bias=extra_bias)
    return rs


def _force_single_act_set():
    """All ACT functions we use (Exp, Ln, Square, Copy) live in the
    natural_log_exp_and_others set.  Empty out every other set so the
    table-load pass resolves everything to one set and a single load
    suffices."""
    import concourse.hw_specs as hw_specs

    orig = hw_specs.get_activation_tables
    target = "natural_log_exp_and_others"

    def patched(arch):
        t = dict(orig(arch))
        need = {AF.Exp, AF.Ln, AF.Square, AF.Copy}
        if target in t and need <= set(t[target]):
            t = {k: (v if k == target else set()) for k, v in t.items()}
        return t

    bacc.get_activation_tables = patched


def build_program():
    _force_single_act_set()
    nc = bacc.Bacc()
    # X^T per batch: [n, d] bf16 (host pre-transposed)
    xt_ext = nc.declare_dram_parameter("XT", [B_LOC, N, D], BF16, isOutput=False)
    # X natural fp8, d-tiles packed in pairs: [kp, p, t, n] with
    # d = kp*256 + t*128 + p
    x8_ext = nc.declare_dram_parameter("X8", [B_LOC, 2, 128, 2, N], F8,
                                       isOutput=False)
    # D_init^T: [r, d] bf16 (host pre-transposed)
    dt_ext = nc.declare_dram_parameter("DT", [B_LOC, R, D], BF16, isOutput=False)
    id_ext = nc.declare_dram_parameter("ident", [128, 128], BF16, isOutput=False)
    y_ext = nc.declare_dram_parameter("Y", [B_LOC, D, N], BF16, isOutput=True)

    with tile.TileContext(nc) as tc:
        with contextlib.ExitStack() as ctx:
            singles = ctx.enter_context(tc.tile_pool(name="singles", bufs=1))
            xpool = ctx.enter_context(tc.tile_pool(name="xpool", bufs=2))
            work = ctx.enter_context(tc.tile_pool(name="work", bufs=2))
            ypool = ctx.enter_context(tc.tile_pool(name="ypool", bufs=8))
            ps_cos = ctx.enter_context(
                tc.tile_pool(name="ps_cos", bufs=3, space="PSUM"))
            ps_m = ctx.enter_context(
                tc.tile_pool(name="ps_m", bufs=1, space="PSUM"))
            ps_out = ctx.enter_context(
                tc.tile_pool(name="ps_out", bufs=2, space="PSUM"))

            id_b = singles.tile([128, 128], BF16)
            nc.sync.dma_start(out=id_b, in_=id_ext[:])
            eps_t = singles.tile([128, 1], F32)
            nc.vector.memset(eps_t, EPS2)
            lnt_t = singles.tile([128, 1], F32)
            nc.vector.memset(lnt_t, LN_TINV)

            def emit_loads(b):
                """Input DMAs for batch b; returns the tiles."""
                xt = xpool.tile([128, NC, D], BF16, tag="xt", name=f"xt{b}")
                # one DMA for the whole [n, d] slab: partition p <-> n-row
                # c*128+p
                nc.sync.dma_start(
                    out=xt,
                    in_=xt_ext[b, :, :].rearrange("(c p) d -> p c d", p=128),
                )
                x8 = []
                for kp in range(2):
                    t = xpool.tile([128, 2, N], F8, tag=f"x8_{kp}",
                                   name=f"x8_{b}_{kp}")
                    nc.sync.dma_start(out=t, in_=x8_ext[b, kp])
                    x8.append(t)
                dt0 = xpool.tile([64, D], BF16, tag="dt0", name=f"dt0_{b}")
                nc.sync.dma_start(out=dt0, in_=dt_ext[b])
                return xt, x8, dt0

            tiles = {0: emit_loads(0)}

            for b in range(B_LOC):
                xt, x8, dt0 = tiles.pop(b)

                # ---- logit scales: scl2[p, c] = T_INV/max(||x_n||, eps) ----
                ssq = work.tile([128, NC], F32, tag="ssq")
                sq_scr = work.tile([128, D], BF16, tag="sqscr", bufs=1)
                for c in range(NC):
                    nc.vector.scalar_tensor_tensor(
                        out=sq_scr, in0=xt[:, c, :], scalar=1.0,
                        in1=xt[:, c, :], op0=OP.mult, op1=OP.mult,
                        accum_out=ssq[:, c:c + 1],
                    )
                ln_x = work.tile([128, NC], F32, tag="lnx")
                nc.scalar.activation(out=ln_x, in_=ssq, func=AF.Ln,
                                     scale=1.0, bias=eps_t[:, 0:1])
                scl2 = work.tile([128, NC], F32, tag="scl2")
                nc.scalar.activation(out=scl2, in_=ln_x, func=AF.Exp,
                                     scale=-0.5, bias=lnt_t[:, 0:1])

                dt_cur = dt0  # [64, D] bf16 (SBUF for s=0, PSUM after)

                for s in range(STEPS):
                    last = s == STEPS - 1
                    # ---- normalize D columns (rows of DT) -> dn8 ----------
                    ssqd = work.tile([64, 1], F32, tag="ssqd")
                    dsq_scr = work.tile([64, D], BF16, tag="dsqscr", bufs=1)
                    nc.scalar.activation(
                        out=dsq_scr, in_=dt_cur, func=AF.Square, scale=1.0,
                        bias=0.0, accum_out=ssqd,
                    )
                    rnd = _rsqrt_clamped(nc, work, ssqd[:, :], 64, "rnd", eps_t)
                    dnt = work.tile([64, D], BF16, tag="dnt")
                    nc.scalar.activation(out=dnt, in_=dt_cur, func=AF.Copy,
                                         scale=rnd[:, 0:1])
                    pdn = ps_m.tile([128, KT, R], BF16, tag="psmall")
                    for k in range(KT):
                        nc.tensor.transpose(
                            pdn[:, k, :], dnt[:, k * 128:(k + 1) * 128],
                            id_b[0:64, 0:64])
                    dn8 = work.tile([128, KT, R], F8, tag="dn8")
                    nc.vector.tensor_copy(out=dn8, in_=pdn)

                    # ---- cos^T via fp8 DoubleRow: [n128, r] per chunk ------
                    pcos = []
                    for g in range(4):
                        pc = ps_cos.tile([128, 8, R], F32, tag="pcos")
                        for j in range(8):
                            c = g * 8 + j
                            for kp in range(2):
                                nc.tensor.matmul(
                                    pc[:, j, :],
                                    x8[kp][:, :, c * 128:(c + 1) * 128],
                                    dn8[:, 2 * kp:2 * kp + 2, :],
                                    start=(kp == 0), stop=(kp == 1),
                                    perf_mode=DR,
                                )
                        pcos.append(pc)

                    # ---- softmax over r (free dim) -------------------------
                    lg = work.tile([128, NC, R], BF16, tag="lg")
                    for g in range(4):
                        nc.vector.tensor_tensor(
                            out=lg[:, 8 * g:8 * (g + 1), :], in0=pcos[g],
                            in1=_bcast_mid(scl2, 8 * g, 8 * (g + 1), R),
                            op=OP.mult,
                        )
                    et = work.tile([128, NC, R], BF16, tag="et")
                    nc.scalar.activation(out=et, in_=lg, func=AF.Exp,
                                         scale=1.0, bias=0.0)
                    s_sum = work.tile([128, NC], F32, tag="ssum")
                    nc.vector.tensor_reduce(
                        out=s_sum, in_=et, axis=mybir.AxisListType.X, op=OP.add)
                    rs = work.tile([128, NC], F32, tag="rs")
                    nc.vector.reciprocal(out=rs, in_=s_sum)
                    rs_b = work.tile([128, NC], BF16, tag="rsb")
                    nc.vector.tensor_copy(out=rs_b, in_=rs)
                    ct = work.tile([128, NC, R], BF16, tag="ct")
                    nc.vector.tensor_tensor(
                        out=ct, in0=et, in1=_bcast_mid(rs_b, 0, NC, R),
                        op=OP.mult,
                    )

                    if b + 1 < B_LOC and s == 0:
                        # prefetch next batch's inputs ahead of this batch's
                        # output DMAs in the queue
                        tiles[b + 1] = emit_loads(b + 1)

                    # ---- XCt = X @ C^T, natural [d128, r] layout -----------
                    pxct = ps_m.tile([128, KT, R], F32, tag="psmall")
                    for dd in range(KT):
                        for c in range(NC):
                            nc.tensor.matmul(
                                pxct[:, dd, :],
                                xt[:, c, dd * 128:(dd + 1) * 128],
                                ct[:, c, :],
                                start=(c == 0), stop=(c == NC - 1),
                            )
                    xct_n = work.tile([128, KT, R], BF16, tag="xctn")
                    nc.vector.tensor_copy(out=xct_n, in_=pxct)

                    # D^T of the new codebook: transpose XCt
                    pdt = ps_m.tile([64, D], BF16, tag="pdt")
                    for dd in range(KT):
                        nc.tensor.transpose(
                            pdt[:, dd * 128:(dd + 1) * 128],
                            xct_n[:, dd, :], id_b)
                    if not last:
                        dt_cur = pdt
                    else:
                        # Dnew = normalize(XCt) in bf16 for the Xbar matmul
                        ssqf = work.tile([64, 1], F32, tag="ssqf")
                        fsq_scr = work.tile([64, D], BF16, tag="fsqscr",
                                            bufs=1)
                        nc.scalar.activation(
                            out=fsq_scr, in_=pdt, func=AF.Square, scale=1.0,
                            bias=0.0, accum_out=ssqf,
                        )
                        rnf = _rsqrt_clamped(nc, work, ssqf[:, :], 64, "rnf",
                                             eps_t)
                        dnew_b = work.tile([64, D], BF16, tag="dnewb")
                        nc.scalar.activation(out=dnew_b, in_=pdt, func=AF.Copy,
                                             scale=rnf[:, 0:1])
                        # C natural [r, n] for Xbar, from transposing ct
                        c_r = work.tile([64, N], BF16, tag="c_r", bufs=1)
                        for q in range(4):
                            pcr = ps_m.tile([64, 8, 128], BF16, tag="pcr")
                            for j in range(8):
                                nc.tensor.transpose(
                                    pcr[:, j, :], ct[:, q * 8 + j, :], id_b)
                            nc.vector.tensor_copy(
                                out=c_r[:, q * 1024:(q + 1) * 1024], in_=pcr)
                        # Xbar = Dnew @ C, write out as bf16
                        yi = 0
                        for k in range(KT):
                            for j in range(N // 512):
                                pxb = ps_out.tile([128, 512], F32, tag="pxb")
                                nc.tensor.matmul(
                                    pxb, dnew_b[:, k * 128:(k + 1) * 128],
                                    c_r[:, j * 512:(j + 1) * 512],
                                    start=True, stop=True,
                                )
                                ot = ypool.tile([128, 512], BF16, tag="osb")
                                if yi % 4 < 2:
                                    nc.scalar.copy(out=ot, in_=pxb)
                                elif yi % 4 == 2:
                                    nc.gpsimd.tensor_copy(out=ot, in_=pxb)
                                else:
                                    nc.vector.tensor_copy(out=ot, in_=pxb)
                                yi += 1
                                nc.sync.dma_start(
                                    out=y_ext[b, k * 128:(k + 1) * 128,
                                              j * 512:(j + 1) * 512],
                                    in_=ot,
                                )
    nc.finalize()
    return nc


_NC_CACHE = None
_last_in_maps = None


def kernel(X: np.ndarray, D_init: np.ndarray) -> np.ndarray:
    global _NC_CACHE, _last_in_maps
    import ml_dtypes

    X = np.asarray(X, dtype=np.float32)
    D_init = np.asarray(D_init, dtype=np.float32)
    if _NC_CACHE is None:
        _NC_CACHE = build_program()
    nc = _NC_CACHE
    ident = np.eye(128, dtype=ml_dtypes.bfloat16)
    xt_h = np.ascontiguousarray(
        X.transpose(0, 2, 1)).astype(ml_dtypes.bfloat16)
    x8_h = np.ascontiguousarray(
        X.reshape(B_FULL, 2, 2, 128, N).transpose(0, 1, 3, 2, 4)
    ).astype(ml_dtypes.float8_e4m3)
    dt_h = np.ascontiguousarray(
        D_init.transpose(0, 2, 1)).astype(ml_dtypes.bfloat16)
    in_maps = [
        {
            "XT": np.ascontiguousarray(xt_h[i * B_LOC:(i + 1) * B_LOC]),
            "X8": np.ascontiguousarray(x8_h[i * B_LOC:(i + 1) * B_LOC]),
            "DT": np.ascontiguousarray(dt_h[i * B_LOC:(i + 1) * B_LOC]),
            "ident": ident,
        }
        for i in range(N_CORES)
    ]
    _last_in_maps = in_maps
    res = run_bass_kernel_spmd(nc, in_maps, list(range(N_CORES)))
    out = np.concatenate(
        [np.asarray(res.results[i]["Y"]) for i in range(N_CORES)], axis=0)
    return out.astype(np.float32)
